# revision 34
# baseline (speedup 1.0000x reference)
"""MAM dense kernel for Trainium2 (8 NeuronCores, SPMD data-parallel over M).

C[m,n] = max_k(x[m,k]*w[n,k]) + min_k(x[m,k]*w[n,k]) + bias[n]

Strategy per core (M_c = 512 rows of x):
  - Layout: n on partitions (8 tiles of 128 n's), k on the free axis.
  - A custom DVE op fuses the whole per-(m,n) chain into ONE streaming
    pass over k:
        body = scan(MAX, w*x) + scan(MIN, w*x) + bias
    The last element of the written stream is exactly max_k + min_k +
    bias; a tiny strided copy per row-group gathers those columns.
  - A hand-authored 2x_1P uop program (registered alongside the 1x
    lowering) processes TWO fp16 elements per cycle: per cycle it forms
    q0/q1, folds them via pairwise MAX/MIN into the two scan
    accumulators, and writes the running result. The engine's perf-mode
    detection engages it when in0/in1/out are fp16, step 1, 4B-aligned
    (perf_max=1 on the instruction unlocks the slot). In either mode
    the final answer is at out[:, K-1], so a detection fallback only
    costs speed, never correctness.
  - x rows are broadcast to all 128 partitions via a stride-0 DMA from
    a fp16 DRAM scratch copy (J rows per DMA, triple-buffered).

MODE:
  "fp16_2x_pgm" — fp16, 2 elem/cycle, paged over m-rows: one
                  instruction covers S m-rows of one n-tile; a
                  hand-authored step state re-seeds the scans at page
                  boundaries (default)
  "fp16_2x_pg" — fp16, 2 elem/cycle, paged over the 8 n-tiles
  "fp16_2x" — fp16 operands, 2 elem/cycle, one instruction per (m, tile)
  "fp32_1x" — fp32 operands, 1 elem/cycle, bit-exact vs reference
"""

import os
import sys

sys.path.insert(0, "/opt/trn_rl_repo")

import numpy as np

M, K, N = 4096, 1024, 1024
N_CORES = 8
M_C = M // N_CORES  # 512 rows per core
NT = N // 128  # 8 n-tiles
J = 4  # m-rows per broadcast DMA / scratch group (non-pgm modes)
S = 16  # m-rows per instruction (pgm mode: pages = m-rows)

MODE = "fp16_2x_pgm"

_last_results = None  # BassKernelResults from the most recent run (for test.py)


def _build_uops_2x():
    """2x_1P datapath: per cycle q0=src0*src1 (lo halves), q1=hi halves,
    pairwise MAX/MIN, scan feedback on both, sum + bias."""
    from concourse.dve_uop import (
        AluInp,
        AluOp,
        DelayInp,
        InpSel,
        OutPath,
        OutSel,
        Trigger,
        UopConfig,
    )

    # input lanes: 0=SRC_0 (blk0 PREV_ALU_OUT), chains: 0=SRC_1, 1=SRC_0_HI,
    # 2=SRC_1_HI, 3=CONST_0, 4=MAX_NEG, 5=ZERO
    seed = UopConfig()
    seed.enable_input(InpSel.MAX_NEG, 5)
    seed.enable_input(InpSel.ZERO, 6)
    for b in range(4):
        seed.datapath_config[b].pass_through_alu()
        seed.datapath_config[b].pass_through_delay(4, 5)
    seed.datapath_config[4].enable_alu(AluOp.BYPASS, AluInp.PREV_DELAY_4)
    seed.datapath_config[4].pass_through_delay(4, 5)
    seed.datapath_config[5].enable_alu(
        AluOp.SUBTRACT, AluInp.PREV_DELAY_5, AluInp.PREV_DELAY_4
    )
    seed.datapath_config[6].pass_through_alu()
    seed.datapath_config[7].pass_through_alu()
    seed.trigger = (Trigger.COUNT, Trigger.NONE, Trigger.NONE)
    seed.repeat_count = 1
    seed.next_uop = (1, 0, 0)

    st = UopConfig()
    st.enable_input(InpSel.SRC_0, 0)
    st.enable_input(InpSel.SRC_1, 1)
    st.enable_input(InpSel.SRC_0_HI, 2)
    st.enable_input(InpSel.SRC_1_HI, 3)
    st.enable_input(InpSel.CONST_0, 4)
    d = st.datapath_config
    d[0].enable_alu(AluOp.MULTIPLY, AluInp.PREV_ALU_OUT, AluInp.PREV_DELAY_0)
    d[0].pass_through_delay(1, 2, 3)
    d[1].enable_alu(AluOp.MULTIPLY, AluInp.PREV_DELAY_1, AluInp.PREV_DELAY_2)
    d[1].enable_delay_from_src(DelayInp.PREV_ALU_OUT, 0)  # q0
    d[1].pass_through_delay(3)
    d[2].enable_alu(AluOp.MAX, AluInp.PREV_ALU_OUT, AluInp.PREV_DELAY_0)
    d[2].pass_through_delay(0, 3)
    d[2].enable_delay_from_src(DelayInp.PREV_ALU_OUT, 1)  # q1
    d[3].enable_alu(AluOp.MIN, AluInp.PREV_DELAY_0, AluInp.PREV_DELAY_1)
    d[3].enable_delay_from_src(DelayInp.PREV_ALU_OUT, 0)  # pairmax
    d[3].pass_through_delay(3)
    d[4].enable_alu(AluOp.MAX, AluInp.CURR_ALU_OUT, AluInp.PREV_DELAY_0)
    d[4].enable_delay_from_src(DelayInp.PREV_ALU_OUT, 0)  # pairmin
    d[4].pass_through_delay(3)
    d[5].enable_alu(AluOp.MIN, AluInp.CURR_ALU_OUT, AluInp.PREV_DELAY_0)
    d[5].enable_delay_from_src(DelayInp.PREV_ALU_OUT, 0)  # scanmax
    d[5].pass_through_delay(3)
    d[6].enable_alu(AluOp.ADD, AluInp.PREV_DELAY_0, AluInp.PREV_ALU_OUT)
    d[6].pass_through_delay(3)
    d[7].enable_alu(AluOp.ADD, AluInp.PREV_ALU_OUT, AluInp.PREV_DELAY_3)
    st.enable_output(OutSel.ALU_OUT, OutPath.WR0_LO)
    st.trigger = (Trigger.SRC_TENSOR_DONE, Trigger.NONE, Trigger.NONE)
    st.next_uop = (0, 0, 0)
    st.require_inp0 = 1
    st.require_inp1 = 1
    return [seed, st]


def _make_paged(states, scan_stages):
    """[seed, steady] -> [seed, steady', step]: steady' detours to the
    step state at SUB_DIM_DONE; step re-seeds the scan accumulators from
    this cycle's pair/product value (ignoring CURR) while consuming
    normally, then returns to steady. scan_stages maps block idx ->
    AluInp the reseed should BYPASS from (the scan's non-CURR operand)."""
    import copy

    from concourse.dve_uop import AluOp, Trigger

    seed, steady = copy.deepcopy(states)
    steady.trigger = (Trigger.SRC_TENSOR_DONE, Trigger.SUB_DIM_DONE, Trigger.NONE)
    steady.next_uop = (0, 2, 0)
    step = copy.deepcopy(steady)
    for blk, src in scan_stages.items():
        b = step.datapath_config[blk]
        b.op = AluOp.BYPASS
        b.alu_src0 = src
        b.alu_src1 = src
    step.trigger = (Trigger.SRC_TENSOR_DONE, Trigger.SUB_DIM_DONE, Trigger.COUNT)
    step.next_uop = (0, 2, 1)
    step.repeat_count = 1
    return [seed, steady, step]


def _register_mam_op():
    """Register the fused scan(MAX)+scan(MIN)+bias DVE op (idempotent),
    with the hand-authored 2x_1P program attached."""
    import concourse.dve_ops as dvo
    from concourse.dve_spec import (
        C0,
        MaxNeg,
        Spec,
        Src0,
        Src1,
        Zero,
        _has_src1,
        lower,
        scan,
    )
    from concourse.dve_uop import AluOp, DveOpSpec

    name = "MAM_SCAN_ANT"
    for op in dvo.OPS:
        if op.name == name:
            return op

    q = Src0 * Src1
    body = scan(AluOp.MAX, q) + scan(AluOp.MIN, q, init=Zero - MaxNeg) + C0

    def _ref(in0, in1, c0, c1, c2):
        P = in0.shape[0]
        qq = in0.astype(np.float32).reshape(P, -1) * np.broadcast_to(
            in1, in0.shape
        ).astype(np.float32).reshape(P, -1)
        r = np.maximum.accumulate(qq, -1) + np.minimum.accumulate(qq, -1)
        c0v = np.asarray(c0, np.float32).reshape(-1, 1)
        return (r + c0v).reshape(in0.shape)

    spec = Spec(body=body, reference=_ref)

    class MamDveOp(dvo.DveOp):
        def compile(self, ver):
            key = (self.name, ver)
            if (r := dvo._COMPILE_CACHE.get(key)) is not None:
                return r
            result = DveOpSpec(
                name=self.name,
                opcode=dvo.get_dve_sub_opcode(self.name),
                uops=lower(self.spec, ver=ver),
                rd1_en=_has_src1(self.spec),
                uops_2x=_build_uops_2x(),
                perf_max=1,
            )
            result.validate(ver)
            dvo._COMPILE_CACHE[key] = result
            return result

    row = dvo._CUSTOM_DVE_ROW_BASE + len(dvo.OPS)
    dvo._SUB_OPCODE_FOR_NAME[name] = row
    op = MamDveOp(name, spec, subdim=False, uops_sha={})
    dvo.OPS.append(op)
    dvo.CUSTOM_DVE_SPECS[name] = spec
    return op


def _register_mam_pg_op():
    """Paged variant: scans reset at each [P,S,N] page boundary, so one
    instruction covers S independent reductions (bias NOT fused; s0=0)."""
    import concourse.dve_ops as dvo
    from concourse.dve_spec import (
        C0,
        MaxNeg,
        Spec,
        Src0,
        Src1,
        Zero,
        _has_src1,
        lower,
        scan,
    )
    from concourse.dve_uop import AluInp, AluOp, DveOpSpec

    name = "MAM_PG_ANT"
    for op in dvo.OPS:
        if op.name == name:
            return op

    q = Src0 * Src1
    body = scan(AluOp.MAX, q) + scan(AluOp.MIN, q, init=Zero - MaxNeg) + C0

    def _ref(in0, in1, c0, c1, c2):
        P = in0.shape[0]
        shp = in0.shape if len(in0.shape) == 3 else (P, 1, -1)
        qq = (
            in0.astype(np.float32).reshape(shp)
            * np.broadcast_to(in1, in0.shape).astype(np.float32).reshape(shp)
        )
        r = np.maximum.accumulate(qq, -1) + np.minimum.accumulate(qq, -1)
        c0v = np.asarray(c0, np.float32).reshape(-1, 1, 1)
        return (r + c0v).reshape(in0.shape)

    spec = Spec(body=body, reference=_ref)

    class MamPgDveOp(dvo.DveOp):
        def compile(self, ver):
            key = (self.name, ver)
            if (r := dvo._COMPILE_CACHE.get(key)) is not None:
                return r
            result = DveOpSpec(
                name=self.name,
                opcode=dvo.get_dve_sub_opcode(self.name),
                uops=_make_paged(
                    lower(self.spec, ver=ver),
                    {1: AluInp.PREV_ALU_OUT, 2: AluInp.PREV_DELAY_0},
                ),
                rd1_en=_has_src1(self.spec),
                uops_2x=_make_paged(
                    _build_uops_2x(),
                    {4: AluInp.PREV_DELAY_0, 5: AluInp.PREV_DELAY_0},
                ),
                perf_max=1,
            )
            result.validate(ver)
            dvo._COMPILE_CACHE[key] = result
            return result

    row = dvo._CUSTOM_DVE_ROW_BASE + len(dvo.OPS)
    dvo._SUB_OPCODE_FOR_NAME[name] = row
    op = MamPgDveOp(name, spec, subdim=True, uops_sha={})
    dvo.OPS.append(op)
    dvo.CUSTOM_DVE_SPECS[name] = spec
    return op


def _emit_mam(nc, op, *, out, in0, in1, s0, perf_max):
    """nc.vector._custom_dve specialized for the MAM op + perf_max."""
    import concourse.bass_isa as bass_isa
    import concourse.mybir as mybir
    from concourse.dve_ops import get_dve_sub_opcode

    v = nc.vector
    if op.name not in v.bass.m.ant_custom_dve_ops:
        v.bass.m.ant_custom_dve_ops = sorted(
            {*v.bass.m.ant_custom_dve_ops, op.name}
        )
    shape = (
        bass_isa.CustomDveShape.STT
        if (in1 is not None and len(in1.shape) > 2)
        else bass_isa.CustomDveShape.TTSS
    )
    isa_opcode = v.bass.isa.Opcode[
        f"NEURON_ISA_TPB_OPCODE_CUSTOM_DVE_ANT_{shape.slot()}"
    ].value
    opt = not op.subdim
    zero = mybir.ImmediateValue(dtype=mybir.dt.float32, value=0.0)
    if isinstance(s0, float):
        s0_l = mybir.ImmediateValue(dtype=mybir.dt.float32, value=s0)
    else:
        s0_l = v.lower_ap(s0, for_isa=True)
    ins = [
        v.lower_ap(in0, for_isa=True, opt=opt),
        v.lower_ap(in1, for_isa=True, opt=opt),
        s0_l,
        zero,
    ]
    outs = [v.lower_ap(out, for_isa=True, opt=opt)]
    return v.add_instruction(
        bass_isa.InstCustomDveAnt(
            name=v.bass.get_next_instruction_name(),
            op_name=op.name,
            rd1_en=True,
            subdim=0x02 if op.subdim else 0,
            imm2=0.0,
            shape=shape,
            row=get_dve_sub_opcode(op.name),
            isa_opcode=isa_opcode,
            ins=ins,
            outs=outs,
            perf_max=perf_max,
        )
    )


def _build_nc(m_c=M_C, nt=NT, k=K, j=J, mode=None):
    import concourse.bacc as bacc
    import concourse.mybir as mybir
    import concourse.tile as tile
    from contextlib import ExitStack

    mode = mode or MODE
    paged = mode in ("fp16_2x_pg", "fp16_2x_pgm")
    pgm = mode == "fp16_2x_pgm"
    MAM = _register_mam_pg_op() if paged else _register_mam_op()

    f32 = mybir.dt.float32
    f16 = mybir.dt.float16
    in_dt = f32 if mode == "fp32_1x" else f16
    perf = 0 if mode == "fp32_1x" else 1
    n_total = nt * 128
    if pgm:
        j = S
    n_groups = m_c // j

    nc = bacc.Bacc("TRN2", target_bir_lowering=False, debug=False)
    # x/w arrive already converted to the operand dtype by the host
    x_d = nc.dram_tensor("x", [m_c, k], in_dt, kind="ExternalInput").ap()
    w_d = nc.dram_tensor("w", [n_total, k], in_dt, kind="ExternalInput").ap()
    b_d = nc.dram_tensor("b", [n_total], f32, kind="ExternalInput").ap()
    o_d = nc.dram_tensor("o", [n_total, m_c], f32, kind="ExternalOutput").ap()
    xs_d = x_d

    with tile.TileContext(nc) as tc, ExitStack() as ctx:
        p_const = ctx.enter_context(tc.tile_pool(name="const", bufs=1))
        w_sb = p_const.tile([128, nt, k], in_dt)
        b_sb = p_const.tile([128, nt], f32)
        res = p_const.tile([128, nt, m_c], f32)

        # w on the sync queue first; bias on the scalar (ACT) queue —
        # the per-group broadcast halves then fill both queues evenly
        nc.sync.dma_start(w_sb[:], w_d.rearrange("(p t) k -> p t k", t=nt))
        nc.scalar.dma_start(b_sb[:], b_d.rearrange("(p t) -> p t", t=nt))

        p_xb = ctx.enter_context(tc.tile_pool(name="xb", bufs=2 if pgm else 3))
        if pgm:
            p_scr = ctx.enter_context(tc.tile_pool(name="scr", bufs=3))
        else:
            scr = p_const.tile([128, nt, j, k], in_dt)

        for g in range(n_groups):
            # broadcast this group's j rows of x to all partitions,
            # split across the sync and scalar DMA queues for bandwidth
            xb = p_xb.tile([128, j, k], in_dt)
            h = j // 2
            for lo, hi, eng in ((0, h, nc.scalar), (h, j, nc.sync)):
                src = (
                    xs_d[g * j + lo : g * j + hi, :]
                    .rearrange("j k -> (j k)")
                    .unsqueeze(0)
                    .broadcast_to([128, (hi - lo) * k])
                )
                eng.dma_start(
                    xb[:, lo:hi, :].rearrange("p j k -> p (j k)"), src
                )

            if pgm:
                # one instruction per n-tile: pages = the j m-rows
                for t in range(nt):
                    sc = p_scr.tile([128, j, k], in_dt)
                    _emit_mam(
                        nc,
                        MAM,
                        out=sc[:],
                        in0=w_sb[:, t, :].unsqueeze(1).broadcast_to([128, j, k]),
                        in1=xb[:],
                        s0=b_sb[:, t : t + 1],
                        perf_max=perf,
                    )
                    # extraction on ScalarE (own SBUF port) keeps the
                    # Vector queue free for the fused ops
                    nc.scalar.copy(res[:, t, g * j : (g + 1) * j], sc[:, :, k - 1])
            elif paged:
                # one instruction per m-row: pages = the nt tiles
                for jj in range(j):
                    _emit_mam(
                        nc,
                        MAM,
                        out=scr[:, :, jj, :],
                        in0=w_sb[:],
                        in1=xb[:, jj, :].unsqueeze(1).broadcast_to([128, nt, k]),
                        s0=0.0,
                        perf_max=perf,
                    )
                # gather last element of each stream + add bias
                nc.vector.tensor_tensor(
                    res[:, :, g * j : (g + 1) * j],
                    scr[:, :, :, k - 1],
                    b_sb[:].unsqueeze(2).broadcast_to([128, nt, j]),
                    mybir.AluOpType.add,
                )
            else:
                for jj in range(j):
                    for t in range(nt):
                        _emit_mam(
                            nc,
                            MAM,
                            out=scr[:, t, jj, :],
                            in0=w_sb[:, t, :],
                            in1=xb[:, jj, :],
                            s0=b_sb[:, t : t + 1],
                            perf_max=perf,
                        )
                # gather the last stream element of each of the nt*j streams
                nc.vector.tensor_copy(
                    res[:, :, g * j : (g + 1) * j], scr[:, :, :, k - 1]
                )
            if pgm:
                # store each group's output as soon as its extractions
                # land — keeps the tail to one small DMA
                nc.sync.dma_start(
                    o_d.rearrange("(p t) m -> p t m", t=nt)[
                        :, :, g * j : (g + 1) * j
                    ],
                    res[:, :, g * j : (g + 1) * j],
                )
            # store finished output in quarters so the final DMA overlaps
            elif (g + 1) % (n_groups // 4) == 0 and g + 1 < n_groups:
                lo = ((g + 1) // (n_groups // 4) - 1) * (m_c // 4)
                hi = lo + m_c // 4
                nc.sync.dma_start(
                    o_d.rearrange("(p t) m -> p t m", t=nt)[:, :, lo:hi],
                    res[:, :, lo:hi],
                )

        if not pgm:
            lo = 3 * (m_c // 4)
            nc.sync.dma_start(
                o_d.rearrange("(p t) m -> p t m", t=nt)[:, :, lo:],
                res[:, :, lo:],
            )

    nc.compile()
    return nc


def kernel(x: np.ndarray, weight: np.ndarray, bias: np.ndarray) -> np.ndarray:
    global _last_results
    from concourse.bass_utils import run_bass_kernel_spmd

    try:  # NTFF tracing needs antenv.axon_hooks; disable if unavailable
        import antenv.axon_hooks  # noqa: F401
    except ImportError:
        os.environ["BASS_NEVER_TRACE"] = "1"

    in_np = np.float32 if MODE == "fp32_1x" else np.float16
    x = np.ascontiguousarray(x, dtype=in_np)
    weight = np.ascontiguousarray(weight, dtype=in_np)
    bias = np.ascontiguousarray(bias, dtype=np.float32)

    nc = _build_nc()
    core_ids = list(range(N_CORES))
    in_maps = [
        {"x": x[c * M_C : (c + 1) * M_C], "w": weight, "b": bias} for c in core_ids
    ]
    res = run_bass_kernel_spmd(nc, in_maps, core_ids)
    _last_results = res

    out = np.empty((M, N), dtype=np.float32)
    for c in core_ids:
        out[c * M_C : (c + 1) * M_C, :] = res.results[c]["o"].T.astype(np.float32)
    return out


# revision 35
# speedup vs baseline: 1.0015x; 1.0015x over previous
"""MAM dense kernel for Trainium2 (8 NeuronCores, SPMD data-parallel over M).

C[m,n] = max_k(x[m,k]*w[n,k]) + min_k(x[m,k]*w[n,k]) + bias[n]

Strategy per core (M_c = 512 rows of x):
  - Layout: n on partitions (8 tiles of 128 n's), k on the free axis.
  - A custom DVE op fuses the whole per-(m,n) chain into ONE streaming
    pass over k:
        body = scan(MAX, w*x) + scan(MIN, w*x) + bias
    The last element of the written stream is exactly max_k + min_k +
    bias; a tiny strided copy per row-group gathers those columns.
  - A hand-authored 2x_1P uop program (registered alongside the 1x
    lowering) processes TWO fp16 elements per cycle: per cycle it forms
    q0/q1, folds them via pairwise MAX/MIN into the two scan
    accumulators, and writes the running result. The engine's perf-mode
    detection engages it when in0/in1/out are fp16, step 1, 4B-aligned
    (perf_max=1 on the instruction unlocks the slot). In either mode
    the final answer is at out[:, K-1], so a detection fallback only
    costs speed, never correctness.
  - x rows are broadcast to all 128 partitions via a stride-0 DMA from
    a fp16 DRAM scratch copy (J rows per DMA, triple-buffered).

MODE:
  "fp16_2x_pgm" — fp16, 2 elem/cycle, paged over m-rows: one
                  instruction covers S m-rows of one n-tile; a
                  hand-authored step state re-seeds the scans at page
                  boundaries (default)
  "fp16_2x_pg" — fp16, 2 elem/cycle, paged over the 8 n-tiles
  "fp16_2x" — fp16 operands, 2 elem/cycle, one instruction per (m, tile)
  "fp32_1x" — fp32 operands, 1 elem/cycle, bit-exact vs reference
"""

import os
import sys

sys.path.insert(0, "/opt/trn_rl_repo")

import numpy as np

M, K, N = 4096, 1024, 1024
N_CORES = 8
M_C = M // N_CORES  # 512 rows per core
NT = N // 128  # 8 n-tiles
J = 4  # m-rows per broadcast DMA / scratch group (non-pgm modes)
S = 16  # m-rows per instruction (pgm mode: pages = m-rows)

MODE = "fp16_2x_pgm"

_last_results = None  # BassKernelResults from the most recent run (for test.py)


def _build_uops_2x():
    """2x_1P datapath: per cycle q0=src0*src1 (lo halves), q1=hi halves,
    pairwise MAX/MIN, scan feedback on both, sum + bias."""
    from concourse.dve_uop import (
        AluInp,
        AluOp,
        DelayInp,
        InpSel,
        OutPath,
        OutSel,
        Trigger,
        UopConfig,
    )

    # input lanes: 0=SRC_0 (blk0 PREV_ALU_OUT), chains: 0=SRC_1, 1=SRC_0_HI,
    # 2=SRC_1_HI, 3=CONST_0, 4=MAX_NEG, 5=ZERO
    seed = UopConfig()
    seed.enable_input(InpSel.MAX_NEG, 5)
    seed.enable_input(InpSel.ZERO, 6)
    for b in range(4):
        seed.datapath_config[b].pass_through_alu()
        seed.datapath_config[b].pass_through_delay(4, 5)
    seed.datapath_config[4].enable_alu(AluOp.BYPASS, AluInp.PREV_DELAY_4)
    seed.datapath_config[4].pass_through_delay(4, 5)
    seed.datapath_config[5].enable_alu(
        AluOp.SUBTRACT, AluInp.PREV_DELAY_5, AluInp.PREV_DELAY_4
    )
    seed.datapath_config[6].pass_through_alu()
    seed.datapath_config[7].pass_through_alu()
    seed.trigger = (Trigger.COUNT, Trigger.NONE, Trigger.NONE)
    seed.repeat_count = 1
    seed.next_uop = (1, 0, 0)

    st = UopConfig()
    st.enable_input(InpSel.SRC_0, 0)
    st.enable_input(InpSel.SRC_1, 1)
    st.enable_input(InpSel.SRC_0_HI, 2)
    st.enable_input(InpSel.SRC_1_HI, 3)
    st.enable_input(InpSel.CONST_0, 4)
    d = st.datapath_config
    d[0].enable_alu(AluOp.MULTIPLY, AluInp.PREV_ALU_OUT, AluInp.PREV_DELAY_0)
    d[0].pass_through_delay(1, 2, 3)
    d[1].enable_alu(AluOp.MULTIPLY, AluInp.PREV_DELAY_1, AluInp.PREV_DELAY_2)
    d[1].enable_delay_from_src(DelayInp.PREV_ALU_OUT, 0)  # q0
    d[1].pass_through_delay(3)
    d[2].enable_alu(AluOp.MAX, AluInp.PREV_ALU_OUT, AluInp.PREV_DELAY_0)
    d[2].pass_through_delay(0, 3)
    d[2].enable_delay_from_src(DelayInp.PREV_ALU_OUT, 1)  # q1
    d[3].enable_alu(AluOp.MIN, AluInp.PREV_DELAY_0, AluInp.PREV_DELAY_1)
    d[3].enable_delay_from_src(DelayInp.PREV_ALU_OUT, 0)  # pairmax
    d[3].pass_through_delay(3)
    d[4].enable_alu(AluOp.MAX, AluInp.CURR_ALU_OUT, AluInp.PREV_DELAY_0)
    d[4].enable_delay_from_src(DelayInp.PREV_ALU_OUT, 0)  # pairmin
    d[4].pass_through_delay(3)
    d[5].enable_alu(AluOp.MIN, AluInp.CURR_ALU_OUT, AluInp.PREV_DELAY_0)
    d[5].enable_delay_from_src(DelayInp.PREV_ALU_OUT, 0)  # scanmax
    d[5].pass_through_delay(3)
    d[6].enable_alu(AluOp.ADD, AluInp.PREV_DELAY_0, AluInp.PREV_ALU_OUT)
    d[6].pass_through_delay(3)
    d[7].enable_alu(AluOp.ADD, AluInp.PREV_ALU_OUT, AluInp.PREV_DELAY_3)
    st.enable_output(OutSel.ALU_OUT, OutPath.WR0_LO)
    st.trigger = (Trigger.SRC_TENSOR_DONE, Trigger.NONE, Trigger.NONE)
    st.next_uop = (0, 0, 0)
    st.require_inp0 = 1
    st.require_inp1 = 1
    return [seed, st]


def _make_paged(states, scan_stages):
    """[seed, steady] -> [seed, steady', step]: steady' detours to the
    step state at SUB_DIM_DONE; step re-seeds the scan accumulators from
    this cycle's pair/product value (ignoring CURR) while consuming
    normally, then returns to steady. scan_stages maps block idx ->
    AluInp the reseed should BYPASS from (the scan's non-CURR operand)."""
    import copy

    from concourse.dve_uop import AluOp, Trigger

    seed, steady = copy.deepcopy(states)
    steady.trigger = (Trigger.SRC_TENSOR_DONE, Trigger.SUB_DIM_DONE, Trigger.NONE)
    steady.next_uop = (0, 2, 0)
    step = copy.deepcopy(steady)
    for blk, src in scan_stages.items():
        b = step.datapath_config[blk]
        b.op = AluOp.BYPASS
        b.alu_src0 = src
        b.alu_src1 = src
    step.trigger = (Trigger.SRC_TENSOR_DONE, Trigger.SUB_DIM_DONE, Trigger.COUNT)
    step.next_uop = (0, 2, 1)
    step.repeat_count = 1
    return [seed, steady, step]


def _register_mam_op():
    """Register the fused scan(MAX)+scan(MIN)+bias DVE op (idempotent),
    with the hand-authored 2x_1P program attached."""
    import concourse.dve_ops as dvo
    from concourse.dve_spec import (
        C0,
        MaxNeg,
        Spec,
        Src0,
        Src1,
        Zero,
        _has_src1,
        lower,
        scan,
    )
    from concourse.dve_uop import AluOp, DveOpSpec

    name = "MAM_SCAN_ANT"
    for op in dvo.OPS:
        if op.name == name:
            return op

    q = Src0 * Src1
    body = scan(AluOp.MAX, q) + scan(AluOp.MIN, q, init=Zero - MaxNeg) + C0

    def _ref(in0, in1, c0, c1, c2):
        P = in0.shape[0]
        qq = in0.astype(np.float32).reshape(P, -1) * np.broadcast_to(
            in1, in0.shape
        ).astype(np.float32).reshape(P, -1)
        r = np.maximum.accumulate(qq, -1) + np.minimum.accumulate(qq, -1)
        c0v = np.asarray(c0, np.float32).reshape(-1, 1)
        return (r + c0v).reshape(in0.shape)

    spec = Spec(body=body, reference=_ref)

    class MamDveOp(dvo.DveOp):
        def compile(self, ver):
            key = (self.name, ver)
            if (r := dvo._COMPILE_CACHE.get(key)) is not None:
                return r
            result = DveOpSpec(
                name=self.name,
                opcode=dvo.get_dve_sub_opcode(self.name),
                uops=lower(self.spec, ver=ver),
                rd1_en=_has_src1(self.spec),
                uops_2x=_build_uops_2x(),
                perf_max=1,
            )
            result.validate(ver)
            dvo._COMPILE_CACHE[key] = result
            return result

    row = dvo._CUSTOM_DVE_ROW_BASE + len(dvo.OPS)
    dvo._SUB_OPCODE_FOR_NAME[name] = row
    op = MamDveOp(name, spec, subdim=False, uops_sha={})
    dvo.OPS.append(op)
    dvo.CUSTOM_DVE_SPECS[name] = spec
    return op


def _register_mam_pg_op():
    """Paged variant: scans reset at each [P,S,N] page boundary, so one
    instruction covers S independent reductions (bias NOT fused; s0=0)."""
    import concourse.dve_ops as dvo
    from concourse.dve_spec import (
        C0,
        MaxNeg,
        Spec,
        Src0,
        Src1,
        Zero,
        _has_src1,
        lower,
        scan,
    )
    from concourse.dve_uop import AluInp, AluOp, DveOpSpec

    name = "MAM_PG_ANT"
    for op in dvo.OPS:
        if op.name == name:
            return op

    q = Src0 * Src1
    body = scan(AluOp.MAX, q) + scan(AluOp.MIN, q, init=Zero - MaxNeg) + C0

    def _ref(in0, in1, c0, c1, c2):
        P = in0.shape[0]
        shp = in0.shape if len(in0.shape) == 3 else (P, 1, -1)
        qq = (
            in0.astype(np.float32).reshape(shp)
            * np.broadcast_to(in1, in0.shape).astype(np.float32).reshape(shp)
        )
        r = np.maximum.accumulate(qq, -1) + np.minimum.accumulate(qq, -1)
        c0v = np.asarray(c0, np.float32).reshape(-1, 1, 1)
        return (r + c0v).reshape(in0.shape)

    spec = Spec(body=body, reference=_ref)

    class MamPgDveOp(dvo.DveOp):
        def compile(self, ver):
            key = (self.name, ver)
            if (r := dvo._COMPILE_CACHE.get(key)) is not None:
                return r
            result = DveOpSpec(
                name=self.name,
                opcode=dvo.get_dve_sub_opcode(self.name),
                uops=_make_paged(
                    lower(self.spec, ver=ver),
                    {1: AluInp.PREV_ALU_OUT, 2: AluInp.PREV_DELAY_0},
                ),
                rd1_en=_has_src1(self.spec),
                uops_2x=_make_paged(
                    _build_uops_2x(),
                    {4: AluInp.PREV_DELAY_0, 5: AluInp.PREV_DELAY_0},
                ),
                perf_max=1,
            )
            result.validate(ver)
            dvo._COMPILE_CACHE[key] = result
            return result

    row = dvo._CUSTOM_DVE_ROW_BASE + len(dvo.OPS)
    dvo._SUB_OPCODE_FOR_NAME[name] = row
    op = MamPgDveOp(name, spec, subdim=True, uops_sha={})
    dvo.OPS.append(op)
    dvo.CUSTOM_DVE_SPECS[name] = spec
    return op


def _emit_mam(nc, op, *, out, in0, in1, s0, perf_max):
    """nc.vector._custom_dve specialized for the MAM op + perf_max."""
    import concourse.bass_isa as bass_isa
    import concourse.mybir as mybir
    from concourse.dve_ops import get_dve_sub_opcode

    v = nc.vector
    if op.name not in v.bass.m.ant_custom_dve_ops:
        v.bass.m.ant_custom_dve_ops = sorted(
            {*v.bass.m.ant_custom_dve_ops, op.name}
        )
    shape = (
        bass_isa.CustomDveShape.STT
        if (in1 is not None and len(in1.shape) > 2)
        else bass_isa.CustomDveShape.TTSS
    )
    isa_opcode = v.bass.isa.Opcode[
        f"NEURON_ISA_TPB_OPCODE_CUSTOM_DVE_ANT_{shape.slot()}"
    ].value
    opt = not op.subdim
    zero = mybir.ImmediateValue(dtype=mybir.dt.float32, value=0.0)
    if isinstance(s0, float):
        s0_l = mybir.ImmediateValue(dtype=mybir.dt.float32, value=s0)
    else:
        s0_l = v.lower_ap(s0, for_isa=True)
    ins = [
        v.lower_ap(in0, for_isa=True, opt=opt),
        v.lower_ap(in1, for_isa=True, opt=opt),
        s0_l,
        zero,
    ]
    outs = [v.lower_ap(out, for_isa=True, opt=opt)]
    return v.add_instruction(
        bass_isa.InstCustomDveAnt(
            name=v.bass.get_next_instruction_name(),
            op_name=op.name,
            rd1_en=True,
            subdim=0x02 if op.subdim else 0,
            imm2=0.0,
            shape=shape,
            row=get_dve_sub_opcode(op.name),
            isa_opcode=isa_opcode,
            ins=ins,
            outs=outs,
            perf_max=perf_max,
        )
    )


def _build_nc(m_c=M_C, nt=NT, k=K, j=J, mode=None):
    import concourse.bacc as bacc
    import concourse.mybir as mybir
    import concourse.tile as tile
    from contextlib import ExitStack

    mode = mode or MODE
    paged = mode in ("fp16_2x_pg", "fp16_2x_pgm")
    pgm = mode == "fp16_2x_pgm"
    MAM = _register_mam_pg_op() if paged else _register_mam_op()

    f32 = mybir.dt.float32
    f16 = mybir.dt.float16
    in_dt = f32 if mode == "fp32_1x" else f16
    perf = 0 if mode == "fp32_1x" else 1
    n_total = nt * 128
    if pgm:
        j = S
    n_groups = m_c // j

    nc = bacc.Bacc("TRN2", target_bir_lowering=False, debug=False)
    # x/w arrive already converted to the operand dtype by the host
    x_d = nc.dram_tensor("x", [m_c, k], in_dt, kind="ExternalInput").ap()
    w_d = nc.dram_tensor("w", [n_total, k], in_dt, kind="ExternalInput").ap()
    b_d = nc.dram_tensor("b", [n_total], f32, kind="ExternalInput").ap()
    o_d = nc.dram_tensor("o", [n_total, m_c], f32, kind="ExternalOutput").ap()
    xs_d = x_d

    with tile.TileContext(nc) as tc, ExitStack() as ctx:
        p_const = ctx.enter_context(tc.tile_pool(name="const", bufs=1))
        w_sb = p_const.tile([128, nt, k], in_dt)
        b_sb = p_const.tile([128, nt], f32)
        res = p_const.tile([128, nt, m_c], f32)

        # w loads on the scalar (ACT) hardware-DGE queue, in parallel
        # with the x broadcasts on the sync queue
        nc.scalar.dma_start(w_sb[:], w_d.rearrange("(p t) k -> p t k", t=nt))
        nc.sync.dma_start(b_sb[:], b_d.rearrange("(p t) -> p t", t=nt))

        p_xb = ctx.enter_context(tc.tile_pool(name="xb", bufs=2 if pgm else 3))
        if pgm:
            p_scr = ctx.enter_context(tc.tile_pool(name="scr", bufs=3))
        else:
            scr = p_const.tile([128, nt, j, k], in_dt)

        for g in range(n_groups):
            # broadcast this group's j rows of x to all partitions
            xb = p_xb.tile([128, j, k], in_dt)
            src = (
                xs_d[g * j : (g + 1) * j, :]
                .rearrange("j k -> (j k)")
                .unsqueeze(0)
                .broadcast_to([128, j * k])
            )
            nc.sync.dma_start(xb[:].rearrange("p j k -> p (j k)"), src)

            if pgm:
                # one instruction per n-tile: pages = the j m-rows
                for t in range(nt):
                    sc = p_scr.tile([128, j, k], in_dt)
                    _emit_mam(
                        nc,
                        MAM,
                        out=sc[:],
                        in0=w_sb[:, t, :].unsqueeze(1).broadcast_to([128, j, k]),
                        in1=xb[:],
                        s0=b_sb[:, t : t + 1],
                        perf_max=perf,
                    )
                    # extraction on ScalarE (own SBUF port) keeps the
                    # Vector queue free for the fused ops
                    nc.scalar.copy(res[:, t, g * j : (g + 1) * j], sc[:, :, k - 1])
            elif paged:
                # one instruction per m-row: pages = the nt tiles
                for jj in range(j):
                    _emit_mam(
                        nc,
                        MAM,
                        out=scr[:, :, jj, :],
                        in0=w_sb[:],
                        in1=xb[:, jj, :].unsqueeze(1).broadcast_to([128, nt, k]),
                        s0=0.0,
                        perf_max=perf,
                    )
                # gather last element of each stream + add bias
                nc.vector.tensor_tensor(
                    res[:, :, g * j : (g + 1) * j],
                    scr[:, :, :, k - 1],
                    b_sb[:].unsqueeze(2).broadcast_to([128, nt, j]),
                    mybir.AluOpType.add,
                )
            else:
                for jj in range(j):
                    for t in range(nt):
                        _emit_mam(
                            nc,
                            MAM,
                            out=scr[:, t, jj, :],
                            in0=w_sb[:, t, :],
                            in1=xb[:, jj, :],
                            s0=b_sb[:, t : t + 1],
                            perf_max=perf,
                        )
                # gather the last stream element of each of the nt*j streams
                nc.vector.tensor_copy(
                    res[:, :, g * j : (g + 1) * j], scr[:, :, :, k - 1]
                )
            if pgm:
                # store each group's output as soon as its extractions
                # land — keeps the tail to one small DMA
                nc.sync.dma_start(
                    o_d.rearrange("(p t) m -> p t m", t=nt)[
                        :, :, g * j : (g + 1) * j
                    ],
                    res[:, :, g * j : (g + 1) * j],
                )
            # store finished output in quarters so the final DMA overlaps
            elif (g + 1) % (n_groups // 4) == 0 and g + 1 < n_groups:
                lo = ((g + 1) // (n_groups // 4) - 1) * (m_c // 4)
                hi = lo + m_c // 4
                nc.sync.dma_start(
                    o_d.rearrange("(p t) m -> p t m", t=nt)[:, :, lo:hi],
                    res[:, :, lo:hi],
                )

        if not pgm:
            lo = 3 * (m_c // 4)
            nc.sync.dma_start(
                o_d.rearrange("(p t) m -> p t m", t=nt)[:, :, lo:],
                res[:, :, lo:],
            )

    nc.compile()
    return nc


def kernel(x: np.ndarray, weight: np.ndarray, bias: np.ndarray) -> np.ndarray:
    global _last_results
    from concourse.bass_utils import run_bass_kernel_spmd

    try:  # NTFF tracing needs antenv.axon_hooks; disable if unavailable
        import antenv.axon_hooks  # noqa: F401
    except ImportError:
        os.environ["BASS_NEVER_TRACE"] = "1"

    in_np = np.float32 if MODE == "fp32_1x" else np.float16
    x = np.ascontiguousarray(x, dtype=in_np)
    weight = np.ascontiguousarray(weight, dtype=in_np)
    bias = np.ascontiguousarray(bias, dtype=np.float32)

    nc = _build_nc()
    core_ids = list(range(N_CORES))
    in_maps = [
        {"x": x[c * M_C : (c + 1) * M_C], "w": weight, "b": bias} for c in core_ids
    ]
    res = run_bass_kernel_spmd(nc, in_maps, core_ids)
    _last_results = res

    out = np.empty((M, N), dtype=np.float32)
    for c in core_ids:
        out[c * M_C : (c + 1) * M_C, :] = res.results[c]["o"].T.astype(np.float32)
    return out


# revision 38
# speedup vs baseline: 1.0026x; 1.0011x over previous
"""MAM dense kernel for Trainium2 (8 NeuronCores, SPMD data-parallel over M).

C[m,n] = max_k(x[m,k]*w[n,k]) + min_k(x[m,k]*w[n,k]) + bias[n]

Strategy per core (M_c = 512 rows of x):
  - Layout: n on partitions (8 tiles of 128 n's), k on the free axis.
  - A custom DVE op fuses the whole per-(m,n) chain into ONE streaming
    pass over k:
        body = scan(MAX, w*x) + scan(MIN, w*x) + bias
    The last element of the written stream is exactly max_k + min_k +
    bias; a tiny strided copy per row-group gathers those columns.
  - A hand-authored 2x_1P uop program (registered alongside the 1x
    lowering) processes TWO fp16 elements per cycle: per cycle it forms
    q0/q1, folds them via pairwise MAX/MIN into the two scan
    accumulators, and writes the running result. The engine's perf-mode
    detection engages it when in0/in1/out are fp16, step 1, 4B-aligned
    (perf_max=1 on the instruction unlocks the slot). In either mode
    the final answer is at out[:, K-1], so a detection fallback only
    costs speed, never correctness.
  - x rows are broadcast to all 128 partitions via a stride-0 DMA from
    a fp16 DRAM scratch copy (J rows per DMA, triple-buffered).

MODE:
  "fp16_2x_pgm" — fp16, 2 elem/cycle, paged over m-rows: one
                  instruction covers S m-rows of one n-tile; a
                  hand-authored step state re-seeds the scans at page
                  boundaries (default)
  "fp16_2x_pg" — fp16, 2 elem/cycle, paged over the 8 n-tiles
  "fp16_2x" — fp16 operands, 2 elem/cycle, one instruction per (m, tile)
  "fp32_1x" — fp32 operands, 1 elem/cycle, bit-exact vs reference
"""

import os
import sys

sys.path.insert(0, "/opt/trn_rl_repo")

import numpy as np

M, K, N = 4096, 1024, 1024
N_CORES = 8
M_C = M // N_CORES  # 512 rows per core
NT = N // 128  # 8 n-tiles
J = 4  # m-rows per broadcast DMA / scratch group (non-pgm modes)
S = 16  # m-rows per instruction (pgm mode: pages = m-rows)

MODE = "fp16_2x_pgm"

_last_results = None  # BassKernelResults from the most recent run (for test.py)


def _build_uops_2x():
    """2x_1P datapath: per cycle q0=src0*src1 (lo halves), q1=hi halves,
    pairwise MAX/MIN, scan feedback on both, sum + bias."""
    from concourse.dve_uop import (
        AluInp,
        AluOp,
        DelayInp,
        InpSel,
        OutPath,
        OutSel,
        Trigger,
        UopConfig,
    )

    # input lanes: 0=SRC_0 (blk0 PREV_ALU_OUT), chains: 0=SRC_1, 1=SRC_0_HI,
    # 2=SRC_1_HI, 3=CONST_0, 4=MAX_NEG, 5=ZERO
    seed = UopConfig()
    seed.enable_input(InpSel.MAX_NEG, 5)
    seed.enable_input(InpSel.ZERO, 6)
    for b in range(4):
        seed.datapath_config[b].pass_through_alu()
        seed.datapath_config[b].pass_through_delay(4, 5)
    seed.datapath_config[4].enable_alu(AluOp.BYPASS, AluInp.PREV_DELAY_4)
    seed.datapath_config[4].pass_through_delay(4, 5)
    seed.datapath_config[5].enable_alu(
        AluOp.SUBTRACT, AluInp.PREV_DELAY_5, AluInp.PREV_DELAY_4
    )
    seed.datapath_config[6].pass_through_alu()
    seed.datapath_config[7].pass_through_alu()
    seed.trigger = (Trigger.COUNT, Trigger.NONE, Trigger.NONE)
    seed.repeat_count = 1
    seed.next_uop = (1, 0, 0)

    st = UopConfig()
    st.enable_input(InpSel.SRC_0, 0)
    st.enable_input(InpSel.SRC_1, 1)
    st.enable_input(InpSel.SRC_0_HI, 2)
    st.enable_input(InpSel.SRC_1_HI, 3)
    st.enable_input(InpSel.CONST_0, 4)
    d = st.datapath_config
    d[0].enable_alu(AluOp.MULTIPLY, AluInp.PREV_ALU_OUT, AluInp.PREV_DELAY_0)
    d[0].pass_through_delay(1, 2, 3)
    d[1].enable_alu(AluOp.MULTIPLY, AluInp.PREV_DELAY_1, AluInp.PREV_DELAY_2)
    d[1].enable_delay_from_src(DelayInp.PREV_ALU_OUT, 0)  # q0
    d[1].pass_through_delay(3)
    d[2].enable_alu(AluOp.MAX, AluInp.PREV_ALU_OUT, AluInp.PREV_DELAY_0)
    d[2].pass_through_delay(0, 3)
    d[2].enable_delay_from_src(DelayInp.PREV_ALU_OUT, 1)  # q1
    d[3].enable_alu(AluOp.MIN, AluInp.PREV_DELAY_0, AluInp.PREV_DELAY_1)
    d[3].enable_delay_from_src(DelayInp.PREV_ALU_OUT, 0)  # pairmax
    d[3].pass_through_delay(3)
    d[4].enable_alu(AluOp.MAX, AluInp.CURR_ALU_OUT, AluInp.PREV_DELAY_0)
    d[4].enable_delay_from_src(DelayInp.PREV_ALU_OUT, 0)  # pairmin
    d[4].pass_through_delay(3)
    d[5].enable_alu(AluOp.MIN, AluInp.CURR_ALU_OUT, AluInp.PREV_DELAY_0)
    d[5].enable_delay_from_src(DelayInp.PREV_ALU_OUT, 0)  # scanmax
    d[5].pass_through_delay(3)
    d[6].enable_alu(AluOp.ADD, AluInp.PREV_DELAY_0, AluInp.PREV_ALU_OUT)
    d[6].pass_through_delay(3)
    d[7].enable_alu(AluOp.ADD, AluInp.PREV_ALU_OUT, AluInp.PREV_DELAY_3)
    st.enable_output(OutSel.ALU_OUT, OutPath.WR0_LO)
    st.trigger = (Trigger.SRC_TENSOR_DONE, Trigger.NONE, Trigger.NONE)
    st.next_uop = (0, 0, 0)
    st.require_inp0 = 1
    st.require_inp1 = 1
    return [seed, st]


def _make_paged(states, scan_stages):
    """[seed, steady] -> [seed, steady', step]: steady' detours to the
    step state at SUB_DIM_DONE; step re-seeds the scan accumulators from
    this cycle's pair/product value (ignoring CURR) while consuming
    normally, then returns to steady. scan_stages maps block idx ->
    AluInp the reseed should BYPASS from (the scan's non-CURR operand)."""
    import copy

    from concourse.dve_uop import AluOp, Trigger

    seed, steady = copy.deepcopy(states)
    steady.trigger = (Trigger.SRC_TENSOR_DONE, Trigger.SUB_DIM_DONE, Trigger.NONE)
    steady.next_uop = (0, 2, 0)
    step = copy.deepcopy(steady)
    for blk, src in scan_stages.items():
        b = step.datapath_config[blk]
        b.op = AluOp.BYPASS
        b.alu_src0 = src
        b.alu_src1 = src
    step.trigger = (Trigger.SRC_TENSOR_DONE, Trigger.SUB_DIM_DONE, Trigger.COUNT)
    step.next_uop = (0, 2, 1)
    step.repeat_count = 1
    return [seed, steady, step]


def _register_mam_op():
    """Register the fused scan(MAX)+scan(MIN)+bias DVE op (idempotent),
    with the hand-authored 2x_1P program attached."""
    import concourse.dve_ops as dvo
    from concourse.dve_spec import (
        C0,
        MaxNeg,
        Spec,
        Src0,
        Src1,
        Zero,
        _has_src1,
        lower,
        scan,
    )
    from concourse.dve_uop import AluOp, DveOpSpec

    name = "MAM_SCAN_ANT"
    for op in dvo.OPS:
        if op.name == name:
            return op

    q = Src0 * Src1
    body = scan(AluOp.MAX, q) + scan(AluOp.MIN, q, init=Zero - MaxNeg) + C0

    def _ref(in0, in1, c0, c1, c2):
        P = in0.shape[0]
        qq = in0.astype(np.float32).reshape(P, -1) * np.broadcast_to(
            in1, in0.shape
        ).astype(np.float32).reshape(P, -1)
        r = np.maximum.accumulate(qq, -1) + np.minimum.accumulate(qq, -1)
        c0v = np.asarray(c0, np.float32).reshape(-1, 1)
        return (r + c0v).reshape(in0.shape)

    spec = Spec(body=body, reference=_ref)

    class MamDveOp(dvo.DveOp):
        def compile(self, ver):
            key = (self.name, ver)
            if (r := dvo._COMPILE_CACHE.get(key)) is not None:
                return r
            result = DveOpSpec(
                name=self.name,
                opcode=dvo.get_dve_sub_opcode(self.name),
                uops=lower(self.spec, ver=ver),
                rd1_en=_has_src1(self.spec),
                uops_2x=_build_uops_2x(),
                perf_max=1,
            )
            result.validate(ver)
            dvo._COMPILE_CACHE[key] = result
            return result

    row = dvo._CUSTOM_DVE_ROW_BASE + len(dvo.OPS)
    dvo._SUB_OPCODE_FOR_NAME[name] = row
    op = MamDveOp(name, spec, subdim=False, uops_sha={})
    dvo.OPS.append(op)
    dvo.CUSTOM_DVE_SPECS[name] = spec
    return op


def _register_mam_pg_op():
    """Paged variant: scans reset at each [P,S,N] page boundary, so one
    instruction covers S independent reductions (bias NOT fused; s0=0)."""
    import concourse.dve_ops as dvo
    from concourse.dve_spec import (
        C0,
        MaxNeg,
        Spec,
        Src0,
        Src1,
        Zero,
        _has_src1,
        lower,
        scan,
    )
    from concourse.dve_uop import AluInp, AluOp, DveOpSpec

    name = "MAM_PG_ANT"
    for op in dvo.OPS:
        if op.name == name:
            return op

    q = Src0 * Src1
    body = scan(AluOp.MAX, q) + scan(AluOp.MIN, q, init=Zero - MaxNeg) + C0

    def _ref(in0, in1, c0, c1, c2):
        P = in0.shape[0]
        shp = in0.shape if len(in0.shape) == 3 else (P, 1, -1)
        qq = (
            in0.astype(np.float32).reshape(shp)
            * np.broadcast_to(in1, in0.shape).astype(np.float32).reshape(shp)
        )
        r = np.maximum.accumulate(qq, -1) + np.minimum.accumulate(qq, -1)
        c0v = np.asarray(c0, np.float32).reshape(-1, 1, 1)
        return (r + c0v).reshape(in0.shape)

    spec = Spec(body=body, reference=_ref)

    class MamPgDveOp(dvo.DveOp):
        def compile(self, ver):
            key = (self.name, ver)
            if (r := dvo._COMPILE_CACHE.get(key)) is not None:
                return r
            result = DveOpSpec(
                name=self.name,
                opcode=dvo.get_dve_sub_opcode(self.name),
                uops=_make_paged(
                    lower(self.spec, ver=ver),
                    {1: AluInp.PREV_ALU_OUT, 2: AluInp.PREV_DELAY_0},
                ),
                rd1_en=_has_src1(self.spec),
                uops_2x=_make_paged(
                    _build_uops_2x(),
                    {4: AluInp.PREV_DELAY_0, 5: AluInp.PREV_DELAY_0},
                ),
                perf_max=1,
            )
            result.validate(ver)
            dvo._COMPILE_CACHE[key] = result
            return result

    row = dvo._CUSTOM_DVE_ROW_BASE + len(dvo.OPS)
    dvo._SUB_OPCODE_FOR_NAME[name] = row
    op = MamPgDveOp(name, spec, subdim=True, uops_sha={})
    dvo.OPS.append(op)
    dvo.CUSTOM_DVE_SPECS[name] = spec
    return op


def _emit_mam(nc, op, *, out, in0, in1, s0, perf_max):
    """nc.vector._custom_dve specialized for the MAM op + perf_max."""
    import concourse.bass_isa as bass_isa
    import concourse.mybir as mybir
    from concourse.dve_ops import get_dve_sub_opcode

    v = nc.vector
    if op.name not in v.bass.m.ant_custom_dve_ops:
        v.bass.m.ant_custom_dve_ops = sorted(
            {*v.bass.m.ant_custom_dve_ops, op.name}
        )
    shape = (
        bass_isa.CustomDveShape.STT
        if (in1 is not None and len(in1.shape) > 2)
        else bass_isa.CustomDveShape.TTSS
    )
    isa_opcode = v.bass.isa.Opcode[
        f"NEURON_ISA_TPB_OPCODE_CUSTOM_DVE_ANT_{shape.slot()}"
    ].value
    opt = not op.subdim
    zero = mybir.ImmediateValue(dtype=mybir.dt.float32, value=0.0)
    if isinstance(s0, float):
        s0_l = mybir.ImmediateValue(dtype=mybir.dt.float32, value=s0)
    else:
        s0_l = v.lower_ap(s0, for_isa=True)
    ins = [
        v.lower_ap(in0, for_isa=True, opt=opt),
        v.lower_ap(in1, for_isa=True, opt=opt),
        s0_l,
        zero,
    ]
    outs = [v.lower_ap(out, for_isa=True, opt=opt)]
    return v.add_instruction(
        bass_isa.InstCustomDveAnt(
            name=v.bass.get_next_instruction_name(),
            op_name=op.name,
            rd1_en=True,
            subdim=0x02 if op.subdim else 0,
            imm2=0.0,
            shape=shape,
            row=get_dve_sub_opcode(op.name),
            isa_opcode=isa_opcode,
            ins=ins,
            outs=outs,
            perf_max=perf_max,
        )
    )


def _build_nc(m_c=M_C, nt=NT, k=K, j=J, mode=None):
    import concourse.bacc as bacc
    import concourse.mybir as mybir
    import concourse.tile as tile
    from contextlib import ExitStack

    mode = mode or MODE
    paged = mode in ("fp16_2x_pg", "fp16_2x_pgm")
    pgm = mode == "fp16_2x_pgm"
    MAM = _register_mam_pg_op() if paged else _register_mam_op()

    f32 = mybir.dt.float32
    f16 = mybir.dt.float16
    in_dt = f32 if mode == "fp32_1x" else f16
    perf = 0 if mode == "fp32_1x" else 1
    n_total = nt * 128
    if pgm:
        j = S
    n_groups = m_c // j

    nc = bacc.Bacc("TRN2", target_bir_lowering=False, debug=False)
    # x/w arrive already converted to the operand dtype by the host
    x_d = nc.dram_tensor("x", [m_c, k], in_dt, kind="ExternalInput").ap()
    w_d = nc.dram_tensor("w", [n_total, k], in_dt, kind="ExternalInput").ap()
    b_d = nc.dram_tensor("b", [n_total], f32, kind="ExternalInput").ap()
    o_d = nc.dram_tensor("o", [n_total, m_c], f32, kind="ExternalOutput").ap()
    xs_d = x_d

    with tile.TileContext(nc) as tc, ExitStack() as ctx:
        p_const = ctx.enter_context(tc.tile_pool(name="const", bufs=1))
        w_sb = p_const.tile([128, nt, k], in_dt)
        b_sb = p_const.tile([128, nt], f32)
        res = p_const.tile([128, nt, m_c], f32)

        # w loads on the scalar (ACT) hardware-DGE queue, in parallel
        # with the x broadcasts on the sync queue
        nc.scalar.dma_start(w_sb[:], w_d.rearrange("(p t) k -> p t k", t=nt))
        nc.sync.dma_start(b_sb[:], b_d.rearrange("(p t) -> p t", t=nt))

        p_xb = ctx.enter_context(tc.tile_pool(name="xb", bufs=2 if pgm else 3))
        if pgm:
            p_scr = ctx.enter_context(tc.tile_pool(name="scr", bufs=3))
        else:
            scr = p_const.tile([128, nt, j, k], in_dt)

        # First two groups are half-size so the first broadcast (the
        # gate on the first fused op) is half the bytes.
        if pgm:
            sizes = [j // 2, j // 2] + [j] * ((m_c - j) // j)
        else:
            sizes = [j] * n_groups

        m0 = 0
        for g, jg in enumerate(sizes):
            # broadcast this group's jg rows of x to all partitions
            xb = p_xb.tile([128, j, k], in_dt)
            src = (
                xs_d[m0 : m0 + jg, :]
                .rearrange("j k -> (j k)")
                .unsqueeze(0)
                .broadcast_to([128, jg * k])
            )
            nc.sync.dma_start(xb[:, 0:jg, :].rearrange("p j k -> p (j k)"), src)

            if pgm:
                # one instruction per n-tile: pages = the jg m-rows
                for t in range(nt):
                    sc = p_scr.tile([128, j, k], in_dt)
                    _emit_mam(
                        nc,
                        MAM,
                        out=sc[:, 0:jg, :],
                        in0=w_sb[:, t, :].unsqueeze(1).broadcast_to([128, jg, k]),
                        in1=xb[:, 0:jg, :],
                        s0=b_sb[:, t : t + 1],
                        perf_max=perf,
                    )
                    # extraction on ScalarE (own SBUF port) keeps the
                    # Vector queue free for the fused ops
                    nc.scalar.copy(res[:, t, m0 : m0 + jg], sc[:, 0:jg, k - 1])
            elif paged:
                # one instruction per m-row: pages = the nt tiles
                for jj in range(j):
                    _emit_mam(
                        nc,
                        MAM,
                        out=scr[:, :, jj, :],
                        in0=w_sb[:],
                        in1=xb[:, jj, :].unsqueeze(1).broadcast_to([128, nt, k]),
                        s0=0.0,
                        perf_max=perf,
                    )
                # gather last element of each stream + add bias
                nc.vector.tensor_tensor(
                    res[:, :, g * j : (g + 1) * j],
                    scr[:, :, :, k - 1],
                    b_sb[:].unsqueeze(2).broadcast_to([128, nt, j]),
                    mybir.AluOpType.add,
                )
            else:
                for jj in range(j):
                    for t in range(nt):
                        _emit_mam(
                            nc,
                            MAM,
                            out=scr[:, t, jj, :],
                            in0=w_sb[:, t, :],
                            in1=xb[:, jj, :],
                            s0=b_sb[:, t : t + 1],
                            perf_max=perf,
                        )
                # gather the last stream element of each of the nt*j streams
                nc.vector.tensor_copy(
                    res[:, :, g * j : (g + 1) * j], scr[:, :, :, k - 1]
                )
            if pgm:
                # store each group's output as soon as its extractions
                # land — keeps the tail to one small DMA
                nc.sync.dma_start(
                    o_d.rearrange("(p t) m -> p t m", t=nt)[:, :, m0 : m0 + jg],
                    res[:, :, m0 : m0 + jg],
                )
            # store finished output in quarters so the final DMA overlaps
            elif (g + 1) % (n_groups // 4) == 0 and g + 1 < n_groups:
                lo = ((g + 1) // (n_groups // 4) - 1) * (m_c // 4)
                hi = lo + m_c // 4
                nc.sync.dma_start(
                    o_d.rearrange("(p t) m -> p t m", t=nt)[:, :, lo:hi],
                    res[:, :, lo:hi],
                )
            m0 += jg

        if not pgm:
            lo = 3 * (m_c // 4)
            nc.sync.dma_start(
                o_d.rearrange("(p t) m -> p t m", t=nt)[:, :, lo:],
                res[:, :, lo:],
            )

    nc.compile()
    return nc


def kernel(x: np.ndarray, weight: np.ndarray, bias: np.ndarray) -> np.ndarray:
    global _last_results
    from concourse.bass_utils import run_bass_kernel_spmd

    try:  # NTFF tracing needs antenv.axon_hooks; disable if unavailable
        import antenv.axon_hooks  # noqa: F401
    except ImportError:
        os.environ["BASS_NEVER_TRACE"] = "1"

    in_np = np.float32 if MODE == "fp32_1x" else np.float16
    x = np.ascontiguousarray(x, dtype=in_np)
    weight = np.ascontiguousarray(weight, dtype=in_np)
    bias = np.ascontiguousarray(bias, dtype=np.float32)

    nc = _build_nc()
    core_ids = list(range(N_CORES))
    in_maps = [
        {"x": x[c * M_C : (c + 1) * M_C], "w": weight, "b": bias} for c in core_ids
    ]
    res = run_bass_kernel_spmd(nc, in_maps, core_ids)
    _last_results = res

    out = np.empty((M, N), dtype=np.float32)
    for c in core_ids:
        out[c * M_C : (c + 1) * M_C, :] = res.results[c]["o"].T.astype(np.float32)
    return out


# revision 39
# speedup vs baseline: 1.0036x; 1.0010x over previous
"""MAM dense kernel for Trainium2 (8 NeuronCores, SPMD data-parallel over M).

C[m,n] = max_k(x[m,k]*w[n,k]) + min_k(x[m,k]*w[n,k]) + bias[n]

Strategy per core (M_c = 512 rows of x):
  - Layout: n on partitions (8 tiles of 128 n's), k on the free axis.
  - A custom DVE op fuses the whole per-(m,n) chain into ONE streaming
    pass over k:
        body = scan(MAX, w*x) + scan(MIN, w*x) + bias
    The last element of the written stream is exactly max_k + min_k +
    bias; a tiny strided copy per row-group gathers those columns.
  - A hand-authored 2x_1P uop program (registered alongside the 1x
    lowering) processes TWO fp16 elements per cycle: per cycle it forms
    q0/q1, folds them via pairwise MAX/MIN into the two scan
    accumulators, and writes the running result. The engine's perf-mode
    detection engages it when in0/in1/out are fp16, step 1, 4B-aligned
    (perf_max=1 on the instruction unlocks the slot). In either mode
    the final answer is at out[:, K-1], so a detection fallback only
    costs speed, never correctness.
  - x rows are broadcast to all 128 partitions via a stride-0 DMA from
    a fp16 DRAM scratch copy (J rows per DMA, triple-buffered).

MODE:
  "fp16_2x_pgm" — fp16, 2 elem/cycle, paged over m-rows: one
                  instruction covers S m-rows of one n-tile; a
                  hand-authored step state re-seeds the scans at page
                  boundaries (default)
  "fp16_2x_pg" — fp16, 2 elem/cycle, paged over the 8 n-tiles
  "fp16_2x" — fp16 operands, 2 elem/cycle, one instruction per (m, tile)
  "fp32_1x" — fp32 operands, 1 elem/cycle, bit-exact vs reference
"""

import os
import sys

sys.path.insert(0, "/opt/trn_rl_repo")

import numpy as np

M, K, N = 4096, 1024, 1024
N_CORES = 8
M_C = M // N_CORES  # 512 rows per core
NT = N // 128  # 8 n-tiles
J = 4  # m-rows per broadcast DMA / scratch group (non-pgm modes)
S = 16  # m-rows per instruction (pgm mode: pages = m-rows)

MODE = "fp16_2x_pgm"

_last_results = None  # BassKernelResults from the most recent run (for test.py)


def _build_uops_2x():
    """2x_1P datapath: per cycle q0=src0*src1 (lo halves), q1=hi halves,
    pairwise MAX/MIN, scan feedback on both, sum + bias."""
    from concourse.dve_uop import (
        AluInp,
        AluOp,
        DelayInp,
        InpSel,
        OutPath,
        OutSel,
        Trigger,
        UopConfig,
    )

    # input lanes: 0=SRC_0 (blk0 PREV_ALU_OUT), chains: 0=SRC_1, 1=SRC_0_HI,
    # 2=SRC_1_HI, 3=CONST_0, 4=MAX_NEG, 5=ZERO
    seed = UopConfig()
    seed.enable_input(InpSel.MAX_NEG, 5)
    seed.enable_input(InpSel.ZERO, 6)
    for b in range(4):
        seed.datapath_config[b].pass_through_alu()
        seed.datapath_config[b].pass_through_delay(4, 5)
    seed.datapath_config[4].enable_alu(AluOp.BYPASS, AluInp.PREV_DELAY_4)
    seed.datapath_config[4].pass_through_delay(4, 5)
    seed.datapath_config[5].enable_alu(
        AluOp.SUBTRACT, AluInp.PREV_DELAY_5, AluInp.PREV_DELAY_4
    )
    seed.datapath_config[6].pass_through_alu()
    seed.datapath_config[7].pass_through_alu()
    seed.trigger = (Trigger.COUNT, Trigger.NONE, Trigger.NONE)
    seed.repeat_count = 1
    seed.next_uop = (1, 0, 0)

    st = UopConfig()
    st.enable_input(InpSel.SRC_0, 0)
    st.enable_input(InpSel.SRC_1, 1)
    st.enable_input(InpSel.SRC_0_HI, 2)
    st.enable_input(InpSel.SRC_1_HI, 3)
    st.enable_input(InpSel.CONST_0, 4)
    d = st.datapath_config
    d[0].enable_alu(AluOp.MULTIPLY, AluInp.PREV_ALU_OUT, AluInp.PREV_DELAY_0)
    d[0].pass_through_delay(1, 2, 3)
    d[1].enable_alu(AluOp.MULTIPLY, AluInp.PREV_DELAY_1, AluInp.PREV_DELAY_2)
    d[1].enable_delay_from_src(DelayInp.PREV_ALU_OUT, 0)  # q0
    d[1].pass_through_delay(3)
    d[2].enable_alu(AluOp.MAX, AluInp.PREV_ALU_OUT, AluInp.PREV_DELAY_0)
    d[2].pass_through_delay(0, 3)
    d[2].enable_delay_from_src(DelayInp.PREV_ALU_OUT, 1)  # q1
    d[3].enable_alu(AluOp.MIN, AluInp.PREV_DELAY_0, AluInp.PREV_DELAY_1)
    d[3].enable_delay_from_src(DelayInp.PREV_ALU_OUT, 0)  # pairmax
    d[3].pass_through_delay(3)
    d[4].enable_alu(AluOp.MAX, AluInp.CURR_ALU_OUT, AluInp.PREV_DELAY_0)
    d[4].enable_delay_from_src(DelayInp.PREV_ALU_OUT, 0)  # pairmin
    d[4].pass_through_delay(3)
    d[5].enable_alu(AluOp.MIN, AluInp.CURR_ALU_OUT, AluInp.PREV_DELAY_0)
    d[5].enable_delay_from_src(DelayInp.PREV_ALU_OUT, 0)  # scanmax
    d[5].pass_through_delay(3)
    d[6].enable_alu(AluOp.ADD, AluInp.PREV_DELAY_0, AluInp.PREV_ALU_OUT)
    d[6].pass_through_delay(3)
    d[7].enable_alu(AluOp.ADD, AluInp.PREV_ALU_OUT, AluInp.PREV_DELAY_3)
    st.enable_output(OutSel.ALU_OUT, OutPath.WR0_LO)
    st.trigger = (Trigger.SRC_TENSOR_DONE, Trigger.NONE, Trigger.NONE)
    st.next_uop = (0, 0, 0)
    st.require_inp0 = 1
    st.require_inp1 = 1
    return [seed, st]


def _make_paged(states, scan_stages):
    """[seed, steady] -> [seed, steady', step]: steady' detours to the
    step state at SUB_DIM_DONE; step re-seeds the scan accumulators from
    this cycle's pair/product value (ignoring CURR) while consuming
    normally, then returns to steady. scan_stages maps block idx ->
    AluInp the reseed should BYPASS from (the scan's non-CURR operand)."""
    import copy

    from concourse.dve_uop import AluOp, Trigger

    seed, steady = copy.deepcopy(states)
    steady.trigger = (Trigger.SRC_TENSOR_DONE, Trigger.SUB_DIM_DONE, Trigger.NONE)
    steady.next_uop = (0, 2, 0)
    step = copy.deepcopy(steady)
    for blk, src in scan_stages.items():
        b = step.datapath_config[blk]
        b.op = AluOp.BYPASS
        b.alu_src0 = src
        b.alu_src1 = src
    step.trigger = (Trigger.SRC_TENSOR_DONE, Trigger.SUB_DIM_DONE, Trigger.COUNT)
    step.next_uop = (0, 2, 1)
    step.repeat_count = 1
    return [seed, steady, step]


def _register_mam_op():
    """Register the fused scan(MAX)+scan(MIN)+bias DVE op (idempotent),
    with the hand-authored 2x_1P program attached."""
    import concourse.dve_ops as dvo
    from concourse.dve_spec import (
        C0,
        MaxNeg,
        Spec,
        Src0,
        Src1,
        Zero,
        _has_src1,
        lower,
        scan,
    )
    from concourse.dve_uop import AluOp, DveOpSpec

    name = "MAM_SCAN_ANT"
    for op in dvo.OPS:
        if op.name == name:
            return op

    q = Src0 * Src1
    body = scan(AluOp.MAX, q) + scan(AluOp.MIN, q, init=Zero - MaxNeg) + C0

    def _ref(in0, in1, c0, c1, c2):
        P = in0.shape[0]
        qq = in0.astype(np.float32).reshape(P, -1) * np.broadcast_to(
            in1, in0.shape
        ).astype(np.float32).reshape(P, -1)
        r = np.maximum.accumulate(qq, -1) + np.minimum.accumulate(qq, -1)
        c0v = np.asarray(c0, np.float32).reshape(-1, 1)
        return (r + c0v).reshape(in0.shape)

    spec = Spec(body=body, reference=_ref)

    class MamDveOp(dvo.DveOp):
        def compile(self, ver):
            key = (self.name, ver)
            if (r := dvo._COMPILE_CACHE.get(key)) is not None:
                return r
            result = DveOpSpec(
                name=self.name,
                opcode=dvo.get_dve_sub_opcode(self.name),
                uops=lower(self.spec, ver=ver),
                rd1_en=_has_src1(self.spec),
                uops_2x=_build_uops_2x(),
                perf_max=1,
            )
            result.validate(ver)
            dvo._COMPILE_CACHE[key] = result
            return result

    row = dvo._CUSTOM_DVE_ROW_BASE + len(dvo.OPS)
    dvo._SUB_OPCODE_FOR_NAME[name] = row
    op = MamDveOp(name, spec, subdim=False, uops_sha={})
    dvo.OPS.append(op)
    dvo.CUSTOM_DVE_SPECS[name] = spec
    return op


def _register_mam_pg_op():
    """Paged variant: scans reset at each [P,S,N] page boundary, so one
    instruction covers S independent reductions (bias NOT fused; s0=0)."""
    import concourse.dve_ops as dvo
    from concourse.dve_spec import (
        C0,
        MaxNeg,
        Spec,
        Src0,
        Src1,
        Zero,
        _has_src1,
        lower,
        scan,
    )
    from concourse.dve_uop import AluInp, AluOp, DveOpSpec

    name = "MAM_PG_ANT"
    for op in dvo.OPS:
        if op.name == name:
            return op

    q = Src0 * Src1
    body = scan(AluOp.MAX, q) + scan(AluOp.MIN, q, init=Zero - MaxNeg) + C0

    def _ref(in0, in1, c0, c1, c2):
        P = in0.shape[0]
        shp = in0.shape if len(in0.shape) == 3 else (P, 1, -1)
        qq = (
            in0.astype(np.float32).reshape(shp)
            * np.broadcast_to(in1, in0.shape).astype(np.float32).reshape(shp)
        )
        r = np.maximum.accumulate(qq, -1) + np.minimum.accumulate(qq, -1)
        c0v = np.asarray(c0, np.float32).reshape(-1, 1, 1)
        return (r + c0v).reshape(in0.shape)

    spec = Spec(body=body, reference=_ref)

    class MamPgDveOp(dvo.DveOp):
        def compile(self, ver):
            key = (self.name, ver)
            if (r := dvo._COMPILE_CACHE.get(key)) is not None:
                return r
            result = DveOpSpec(
                name=self.name,
                opcode=dvo.get_dve_sub_opcode(self.name),
                uops=_make_paged(
                    lower(self.spec, ver=ver),
                    {1: AluInp.PREV_ALU_OUT, 2: AluInp.PREV_DELAY_0},
                ),
                rd1_en=_has_src1(self.spec),
                uops_2x=_make_paged(
                    _build_uops_2x(),
                    {4: AluInp.PREV_DELAY_0, 5: AluInp.PREV_DELAY_0},
                ),
                perf_max=1,
            )
            result.validate(ver)
            dvo._COMPILE_CACHE[key] = result
            return result

    row = dvo._CUSTOM_DVE_ROW_BASE + len(dvo.OPS)
    dvo._SUB_OPCODE_FOR_NAME[name] = row
    op = MamPgDveOp(name, spec, subdim=True, uops_sha={})
    dvo.OPS.append(op)
    dvo.CUSTOM_DVE_SPECS[name] = spec
    return op


def _emit_mam(nc, op, *, out, in0, in1, s0, perf_max):
    """nc.vector._custom_dve specialized for the MAM op + perf_max."""
    import concourse.bass_isa as bass_isa
    import concourse.mybir as mybir
    from concourse.dve_ops import get_dve_sub_opcode

    v = nc.vector
    if op.name not in v.bass.m.ant_custom_dve_ops:
        v.bass.m.ant_custom_dve_ops = sorted(
            {*v.bass.m.ant_custom_dve_ops, op.name}
        )
    shape = (
        bass_isa.CustomDveShape.STT
        if (in1 is not None and len(in1.shape) > 2)
        else bass_isa.CustomDveShape.TTSS
    )
    isa_opcode = v.bass.isa.Opcode[
        f"NEURON_ISA_TPB_OPCODE_CUSTOM_DVE_ANT_{shape.slot()}"
    ].value
    opt = not op.subdim
    zero = mybir.ImmediateValue(dtype=mybir.dt.float32, value=0.0)
    if isinstance(s0, float):
        s0_l = mybir.ImmediateValue(dtype=mybir.dt.float32, value=s0)
    else:
        s0_l = v.lower_ap(s0, for_isa=True)
    ins = [
        v.lower_ap(in0, for_isa=True, opt=opt),
        v.lower_ap(in1, for_isa=True, opt=opt),
        s0_l,
        zero,
    ]
    outs = [v.lower_ap(out, for_isa=True, opt=opt)]
    return v.add_instruction(
        bass_isa.InstCustomDveAnt(
            name=v.bass.get_next_instruction_name(),
            op_name=op.name,
            rd1_en=True,
            subdim=0x02 if op.subdim else 0,
            imm2=0.0,
            shape=shape,
            row=get_dve_sub_opcode(op.name),
            isa_opcode=isa_opcode,
            ins=ins,
            outs=outs,
            perf_max=perf_max,
        )
    )


def _build_nc(m_c=M_C, nt=NT, k=K, j=J, mode=None):
    import concourse.bacc as bacc
    import concourse.mybir as mybir
    import concourse.tile as tile
    from contextlib import ExitStack

    mode = mode or MODE
    paged = mode in ("fp16_2x_pg", "fp16_2x_pgm")
    pgm = mode == "fp16_2x_pgm"
    MAM = _register_mam_pg_op() if paged else _register_mam_op()

    f32 = mybir.dt.float32
    f16 = mybir.dt.float16
    in_dt = f32 if mode == "fp32_1x" else f16
    perf = 0 if mode == "fp32_1x" else 1
    n_total = nt * 128
    if pgm:
        j = S
    n_groups = m_c // j

    nc = bacc.Bacc("TRN2", target_bir_lowering=False, debug=False)
    # x/w arrive already converted to the operand dtype by the host
    x_d = nc.dram_tensor("x", [m_c, k], in_dt, kind="ExternalInput").ap()
    w_d = nc.dram_tensor("w", [n_total, k], in_dt, kind="ExternalInput").ap()
    b_d = nc.dram_tensor("b", [n_total], f32, kind="ExternalInput").ap()
    o_d = nc.dram_tensor("o", [n_total, m_c], f32, kind="ExternalOutput").ap()
    xs_d = x_d

    with tile.TileContext(nc) as tc, ExitStack() as ctx:
        p_const = ctx.enter_context(tc.tile_pool(name="const", bufs=1))
        w_sb = p_const.tile([128, nt, k], in_dt)
        b_sb = p_const.tile([128, nt], f32)
        res = p_const.tile([128, nt, m_c], f32)

        # w loads on the scalar (ACT) hardware-DGE queue, in parallel
        # with the x broadcasts on the sync queue
        nc.scalar.dma_start(w_sb[:], w_d.rearrange("(p t) k -> p t k", t=nt))
        nc.sync.dma_start(b_sb[:], b_d.rearrange("(p t) -> p t", t=nt))

        p_xb = ctx.enter_context(tc.tile_pool(name="xb", bufs=3))
        if pgm:
            p_scr = ctx.enter_context(tc.tile_pool(name="scr", bufs=2))
        else:
            scr = p_const.tile([128, nt, j, k], in_dt)

        # First two groups are half-size so the first broadcast (the
        # gate on the first fused op) is half the bytes.
        if pgm:
            sizes = [j // 2, j // 2] + [j] * ((m_c - j) // j)
        else:
            sizes = [j] * n_groups

        m0 = 0
        for g, jg in enumerate(sizes):
            # broadcast this group's jg rows of x to all partitions
            xb = p_xb.tile([128, j, k], in_dt)
            src = (
                xs_d[m0 : m0 + jg, :]
                .rearrange("j k -> (j k)")
                .unsqueeze(0)
                .broadcast_to([128, jg * k])
            )
            nc.sync.dma_start(xb[:, 0:jg, :].rearrange("p j k -> p (j k)"), src)

            if pgm:
                # one instruction per n-tile: pages = the jg m-rows
                for t in range(nt):
                    sc = p_scr.tile([128, j, k], in_dt)
                    _emit_mam(
                        nc,
                        MAM,
                        out=sc[:, 0:jg, :],
                        in0=w_sb[:, t, :].unsqueeze(1).broadcast_to([128, jg, k]),
                        in1=xb[:, 0:jg, :],
                        s0=b_sb[:, t : t + 1],
                        perf_max=perf,
                    )
                    # extraction on ScalarE (own SBUF port) keeps the
                    # Vector queue free for the fused ops
                    nc.scalar.copy(res[:, t, m0 : m0 + jg], sc[:, 0:jg, k - 1])
            elif paged:
                # one instruction per m-row: pages = the nt tiles
                for jj in range(j):
                    _emit_mam(
                        nc,
                        MAM,
                        out=scr[:, :, jj, :],
                        in0=w_sb[:],
                        in1=xb[:, jj, :].unsqueeze(1).broadcast_to([128, nt, k]),
                        s0=0.0,
                        perf_max=perf,
                    )
                # gather last element of each stream + add bias
                nc.vector.tensor_tensor(
                    res[:, :, g * j : (g + 1) * j],
                    scr[:, :, :, k - 1],
                    b_sb[:].unsqueeze(2).broadcast_to([128, nt, j]),
                    mybir.AluOpType.add,
                )
            else:
                for jj in range(j):
                    for t in range(nt):
                        _emit_mam(
                            nc,
                            MAM,
                            out=scr[:, t, jj, :],
                            in0=w_sb[:, t, :],
                            in1=xb[:, jj, :],
                            s0=b_sb[:, t : t + 1],
                            perf_max=perf,
                        )
                # gather the last stream element of each of the nt*j streams
                nc.vector.tensor_copy(
                    res[:, :, g * j : (g + 1) * j], scr[:, :, :, k - 1]
                )
            if pgm:
                # store each group's output as soon as its extractions
                # land — keeps the tail to one small DMA
                nc.sync.dma_start(
                    o_d.rearrange("(p t) m -> p t m", t=nt)[:, :, m0 : m0 + jg],
                    res[:, :, m0 : m0 + jg],
                )
            # store finished output in quarters so the final DMA overlaps
            elif (g + 1) % (n_groups // 4) == 0 and g + 1 < n_groups:
                lo = ((g + 1) // (n_groups // 4) - 1) * (m_c // 4)
                hi = lo + m_c // 4
                nc.sync.dma_start(
                    o_d.rearrange("(p t) m -> p t m", t=nt)[:, :, lo:hi],
                    res[:, :, lo:hi],
                )
            m0 += jg

        if not pgm:
            lo = 3 * (m_c // 4)
            nc.sync.dma_start(
                o_d.rearrange("(p t) m -> p t m", t=nt)[:, :, lo:],
                res[:, :, lo:],
            )

    nc.compile()
    return nc


def kernel(x: np.ndarray, weight: np.ndarray, bias: np.ndarray) -> np.ndarray:
    global _last_results
    from concourse.bass_utils import run_bass_kernel_spmd

    try:  # NTFF tracing needs antenv.axon_hooks; disable if unavailable
        import antenv.axon_hooks  # noqa: F401
    except ImportError:
        os.environ["BASS_NEVER_TRACE"] = "1"

    in_np = np.float32 if MODE == "fp32_1x" else np.float16
    x = np.ascontiguousarray(x, dtype=in_np)
    weight = np.ascontiguousarray(weight, dtype=in_np)
    bias = np.ascontiguousarray(bias, dtype=np.float32)

    nc = _build_nc()
    core_ids = list(range(N_CORES))
    in_maps = [
        {"x": x[c * M_C : (c + 1) * M_C], "w": weight, "b": bias} for c in core_ids
    ]
    res = run_bass_kernel_spmd(nc, in_maps, core_ids)
    _last_results = res

    out = np.empty((M, N), dtype=np.float32)
    for c in core_ids:
        out[c * M_C : (c + 1) * M_C, :] = res.results[c]["o"].T.astype(np.float32)
    return out
